# revision 20
# baseline (speedup 1.0000x reference)
"""Trainium2 Bass kernel for nn_CrossChannelAttention.

Reference computation (per batch b, pixel p, with C=128 channels, NUMS=16
groups of HEADS=8 channels, OUT=256):
    fm[g,p]  = relu(sum_h W1[g,h] * x[8g+h, p] + b1[g])          # [16, P]
    feat[(g,d), p] = fm[g,p] * x[d,p]                            # [2048, P]
    out[o,p] = sum_c W2[o,c] * feat[c,p] + b2[o]                 # [256, P]

Strategy: data-parallel over batch B=8 across the 8 NeuronCores (one image
per core, params replicated).  Per core the PE-bound floor is 256 bf16
matmuls [K=128,M=128,N=512] ~= 57us; everything else must hide under it.

v6 (trace history: v2=90.8..105.2 run-variance, v3=100.4, v4=102.5,
v5=98.7; late-session like-for-like: v5/v6 ~98-100 vs v2 ~105):
  - The two HWDGE rings together sustain only ~280GB/s of broadcast SBUF
    writes (16 shared DMA engines; ~60-90GB/s for the first ~10us while
    clocks ramp), vs 280GB/s of rep-broadcast demand at full PE speed.
    gpsimd partition_broadcast carries 7 of the 32 rep units (g0/g7/g11
    of phase A, g0/g5/g9/g13 of phase B; g>0 via [1,1024] p0-row copies,
    since gpsimd only reads partition 0).
  - ALL rep tiles fully SBUF-resident; ring orders hand-scheduled so every
    transfer lands just before its consumer (the HAM clock gate is chip-
    wide: any PE stall downshifts every engine ~2x for ~3-10us).
  - All full-width ft tiles share one 4-buf rotation tag: paces the DVE
    ~4 units ahead of the PE and stops the Tile scheduler from hoisting
    phase-B fts ahead of phase-A-critical ones.
  - Two 2048-px phases x 8 PSUM banks; fmB matmuls interleave into mains
    g=0 (pso_3_* tag rotation: pfB created before psoA); fm relus split
    scalar/DVE; drains chase the PE bank order (scalar=oc0, vector=oc1).
Accuracy: bf16 matmuls with fp32 PSUM accumulation; rel err ~4e-3.
"""

import numpy as np
import ml_dtypes

import concourse.bacc as bacc
import concourse.tile as tile
from concourse import mybir
from concourse.bass_utils import run_bass_kernel_spmd

F32 = mybir.dt.float32
BF16 = mybir.dt.bfloat16

B, C, H, W = 8, 128, 64, 64
NUMS, HEADS, OUT = 16, 8, 256
P = H * W          # 4096 pixels per image
PB = 512           # pixel block (one PSUM bank of fp32)
PH = 2048          # phase width (4 pixel blocks; all 8 PSUM banks)
N_CORES = 8

GPS_A = (7, 11)        # phase-A units on gpsimd (plus g=0)
GPS_B = (5, 9, 13)     # phase-B units on gpsimd (plus g=0)

_CACHE = {}


def _build():
    nc = bacc.Bacc("TRN2", target_bir_lowering=False, debug=False,
                   num_devices=N_CORES)

    x_d = nc.dram_tensor("x", [C, P], BF16, kind="ExternalInput")
    w1s_d = nc.dram_tensor("w1s", [C, 256], BF16, kind="ExternalInput")
    w2t_d = nc.dram_tensor("w2t", [C, NUMS * OUT], BF16, kind="ExternalInput")
    b1_d = nc.dram_tensor("b1c", [NUMS, 128], F32, kind="ExternalInput")
    b2_d = nc.dram_tensor("b2c", [C, 128], F32, kind="ExternalInput")
    out_d = nc.dram_tensor("out", [OUT, P], BF16, kind="ExternalOutput")

    relu = mybir.ActivationFunctionType.Relu
    ident = mybir.ActivationFunctionType.Identity
    mult = mybir.AluOpType.mult
    add = mybir.AluOpType.add
    amax = mybir.AluOpType.max

    def vrelu(out_ap, in_ap, bias_ap):
        nc.vector.tensor_scalar(out_ap, in_ap, bias_ap, 0.0,
                                op0=add, op1=amax)

    with tile.TileContext(nc) as tc:
        with (
            tc.tile_pool(name="const", bufs=1) as cpool,
            tc.tile_pool(name="repA", bufs=1) as repAp,
            tc.tile_pool(name="repB", bufs=1) as repBp,
            tc.tile_pool(name="ft", bufs=1) as ftp,
            tc.tile_pool(name="osb", bufs=1) as osbp,
            tc.tile_pool(name="ps", bufs=1, space="PSUM") as ps,
            tc.tile_pool(name="dr", bufs=1, space="DRAM") as drp,
        ):
            # ---- tiles ----
            scratch = cpool.tile([C, C + PB], BF16)
            nc.vector.memset(scratch[:], 0.0)

            w1s_t = cpool.tile([C, 256], BF16)
            b1_t = cpool.tile([NUMS, 128], F32)
            b2_t = cpool.tile([C, 128], F32)
            xA = cpool.tile([C, PH], BF16, name="xA")
            xB = cpool.tile([C, PH], BF16, name="xB")
            w2c = [cpool.tile([C, PH], BF16, name=f"w2c{j}") for j in range(2)]
            fmhA = cpool.tile([NUMS, PH], BF16, name="fmhA")
            fmhB = cpool.tile([NUMS, PH], BF16, name="fmhB")
            fm_drA = drp.tile([NUMS, PH], BF16, name="fm_drA")
            fm_drB = drp.tile([NUMS, PH], BF16, name="fm_drB")

            # sync: w1s + xA quarters (fm chain + mains g0/g1 via w2c0a)
            nc.sync.dma_start(w1s_t[:], w1s_d[:])
            nc.sync.dma_start(w2c[0][:, 0:PB], w2t_d[:, 0:PB])
            for q in range(4):
                qx = slice(q * PB, (q + 1) * PB)
                nc.sync.dma_start(xA[:, qx], x_d[:, qx])
            # scalar: biases, xB front (fmB), rest of w2c0
            nc.scalar.dma_start(b1_t[:], b1_d[:])
            nc.scalar.dma_start(b2_t[:], b2_d[:])
            nc.scalar.dma_start(xB[:, 0:PB], x_d[:, PH:PH + PB])
            nc.scalar.dma_start(w2c[0][:, PB:PH], w2t_d[:, PB:PH])
            nc.scalar.dma_start(xB[:, PB:PH], x_d[:, PH + PB:P])

            # ---- PSUM: 8 banks, tag-per-bank, serial reuse ----
            def psum(pb, oc, nm, parts=C):
                return ps.tile([parts, PB], F32, tag=f"pso_{pb}_{oc}",
                               name=nm)

            ps_w = psum(0, 0, "ps_warm")

            def warm():
                nc.tensor.matmul(ps_w[:], scratch[:, 0:C],
                                 scratch[:, C:C + PB], start=True, stop=True)

            warm()

            # ---- fm phase A; relus A0/A1/A3 on DVE, A2 on scalar ----
            fm_psA_tags = [(1, 0), (1, 1), (2, 0), (2, 1)]
            pfA = [psum(*fm_psA_tags[i], nm=f"psfmA{i}", parts=NUMS)
                   for i in range(4)]
            rep0q = [repAp.tile([C, PB], BF16, name=f"rep0q{i}")
                     for i in range(4)]
            ft0q = [ftp.tile([C, PB], BF16, name=f"ft0q{i}")
                    for i in range(4)]

            def fmA(i):
                qx = slice(i * PB, (i + 1) * PB)
                nc.tensor.matmul(pfA[i][:], w1s_t[:, 0:NUMS], xA[:, qx],
                                 start=True, stop=True)

            def fmA_relu(i):
                qx = slice(i * PB, (i + 1) * PB)
                if i == 2:
                    nc.scalar.activation(fmhA[:, qx], pfA[i][:], relu,
                                         bias=b1_t[:, 0:1])
                else:
                    vrelu(fmhA[:, qx], pfA[i][:], b1_t[:, 0:1])

            def gps0q(i):
                qx = slice(i * PB, (i + 1) * PB)
                nc.gpsimd.partition_broadcast(rep0q[i][:], fmhA[0:1, qx])

            def ft0(i):
                qx = slice(i * PB, (i + 1) * PB)
                nc.vector.tensor_tensor(ft0q[i][:], xA[:, qx], rep0q[i][:],
                                        op=mult)

            fmA(0)
            fmA_relu(0)          # DVE
            gps0q(0)
            fmA(1)
            fmA_relu(1)          # DVE
            gps0q(1)
            fmA(2)
            fmA_relu(2)          # scalar
            gps0q(2)
            fmA(3)
            ft0(0)               # DVE (before rA3 so mains start earliest)
            ft0(1)
            fmA_relu(3)          # DVE
            gps0q(3)
            # fm mirror halves on SYNC (rep1h depends on them)
            nc.sync.dma_start(fm_drA[:, 0:1024], fmhA[0:NUMS, 0:1024])
            warm()
            warm()
            # rep g1 halves via gpsimd (p0-row copies are tiny, freeing
            # 512KB of early ring bandwidth; gpsimd is free after rep0q)
            repAh = {}
            for g in (1, 2, 3):
                for h in range(2):
                    repAh[(g, h)] = repAp.tile([C, 1024], BF16,
                                               name=f"rep{g}h{h}")
            p0rows = {}

            def p0row(ph, g, h):
                t = cpool.tile([1, 1024], BF16, tag="p0row", bufs=4,
                               name=f"p0r{ph}{g}h{h}")
                p0rows[(ph, g, h)] = t
                return t

            nc.sync.dma_start(p0row("A", 1, 0)[:], fm_drA[1:2, 0:1024])
            nc.sync.dma_start(fm_drA[:, 1024:PH], fmhA[0:NUMS, 1024:PH])
            nc.sync.dma_start(p0row("A", 1, 1)[:], fm_drA[1:2, 1024:PH])
            nc.gpsimd.partition_broadcast(repAh[(1, 0)][:],
                                          p0rows[("A", 1, 0)][:])
            nc.gpsimd.partition_broadcast(repAh[(1, 1)][:],
                                          p0rows[("A", 1, 1)][:])
            for g in GPS_A:
                for h in range(2):
                    hx = slice(h * 1024, (h + 1) * 1024)
                    nc.sync.dma_start(p0row("A", g, h)[:],
                                      fm_drA[g:g + 1, hx])
            nc.sync.dma_start(w2c[1][:, 0:PB], w2t_d[:, PH:PH + PB])
            repAg = {}
            for g in GPS_A:
                r = repAp.tile([C, PH], BF16, name=f"repAg{g}")
                for h in range(2):
                    hx = slice(h * 1024, (h + 1) * 1024)
                    nc.gpsimd.partition_broadcast(r[:, hx],
                                                  p0rows[("A", g, h)][:])
                repAg[g] = r
            # phase-A full reps: sync g4,g5,g6,g8,g10,g12,g14;
            # scalar g9,g13,g15 later
            dmaA = [g for g in range(4, NUMS) if g not in GPS_A]
            repA = {}
            for g in dmaA:
                repA[g] = repAp.tile([C, PH], BF16, tag="repfA",
                                     bufs=len(dmaA), name=f"repA{g}")

            def emit_repA(eng, g):
                eng.dma_start(repA[g][:],
                              fm_drA[g:g + 1, :].broadcast_to((C, PH)))

            for g in (4, 5, 6, 8, 10, 12, 14):
                emit_repA(nc.sync, g)
            # scalar ring: g2/g3 halves (after its loads)
            for g in (2, 3):
                for h in range(2):
                    hx = slice(h * 1024, (h + 1) * 1024)
                    nc.scalar.dma_start(
                        repAh[(g, h)][:],
                        fm_drA[g:g + 1, hx].broadcast_to((C, 1024)))

            # phase-B fm psum tiles before phase-A main accumulators
            fm_psB_tags = [(3, 0), (3, 1), (3, 0), (3, 1)]
            pfB = [psum(*fm_psB_tags[i], nm=f"psfmB{i}", parts=NUMS)
                   for i in range(4)]

            ftA = {}
            ftA[(0, 0)], ftA[(0, 1)] = ft0q[0], ft0q[1]
            ftA[(0, 2)], ftA[(0, 3)] = ft0q[2], ft0q[3]

            def rhsA(g, pb):
                if g == 0:
                    return ftA[(0, pb)][:]
                if g in (1, 2, 3):
                    h, r = divmod(pb, 2)
                    return ftA[(g, h)][:, r * PB:(r + 1) * PB]
                return ftA[g][:, pb * PB:(pb + 1) * PB]

            def w2blk(g, oc):
                j, r = divmod(g, 8)
                cx = slice((r * 2 + oc) * C, (r * 2 + oc + 1) * C)
                return w2c[j][:, cx]

            psoA = {(pb, oc): psum(pb, oc, f"psoA{pb}_{oc}")
                    for pb in range(4) for oc in range(2)}

            def fmB(i):
                qx = slice(i * PB, (i + 1) * PB)
                nc.tensor.matmul(pfB[i][:], w1s_t[:, 0:NUMS], xB[:, qx],
                                 start=True, stop=True)

            def fmB_relu(i):
                qx = slice(i * PB, (i + 1) * PB)
                vrelu(fmhB[:, qx], pfB[i][:], b1_t[:, 0:1])

            # unified full-ft rotation: paces DVE ~4 units ahead of the PE
            # and pins the scheduler to consumption order
            def ftf(nm):
                return ftp.tile([C, PH], BF16, tag="ftf", bufs=4, name=nm)

            for g in range(NUMS):
                for pb in range(4):
                    for oc in range(2):
                        nc.tensor.matmul(psoA[(pb, oc)][:], w2blk(g, oc),
                                         rhsA(g, pb),
                                         start=(g == 0), stop=(g == NUMS - 1))
                    if g == 0 and pb == 0:
                        fmB(0)
                        fmB(1)
                        fmB_relu(0)
                        fmB_relu(1)
                        ft0(2)
                        ft0(3)
                    if g == 0 and pb == 1:
                        fmB(2)
                        fmB(3)
                        fmB_relu(2)
                        fmB_relu(3)
                if g == 0:
                    # DVE next: g1..g3 ft halves
                    for gg in (1, 2, 3):
                        for h in range(2):
                            hx = slice(h * 1024, (h + 1) * 1024)
                            f = ftp.tile([C, 1024], BF16, name=f"ft{gg}h{h}")
                            nc.vector.tensor_tensor(f[:], xA[:, hx],
                                                    repAh[(gg, h)][:],
                                                    op=mult)
                            ftA[(gg, h)] = f
                    # fm_drB mirror early on scalar (tiny), then the rest of
                    # the scalar ring's phase-A work, then phase-B prep
                    nc.scalar.dma_start(fm_drB[:, 0:1024],
                                        fmhB[0:NUMS, 0:1024])
                    nc.scalar.dma_start(fm_drB[:, 1024:PH],
                                        fmhB[0:NUMS, 1024:PH])
                    nc.scalar.dma_start(w2c[1][:, PB:PH],
                                        w2t_d[:, PH + PB:2 * PH])
                    for gg in (9, 13, 15):
                        emit_repA(nc.scalar, gg)
                    rep0B = repBp.tile([C, PH], BF16, name="rep0B")
                    nc.gpsimd.partition_broadcast(rep0B[:], fmhB[0:1, :])
                    repBg = {}
                    for gb in GPS_B:
                        r = repBp.tile([C, PH], BF16, name=f"repBg{gb}")
                        for h in range(2):
                            hx = slice(h * 1024, (h + 1) * 1024)
                            nc.sync.dma_start(p0row("B", gb, h)[:],
                                              fm_drB[gb:gb + 1, hx])
                            nc.gpsimd.partition_broadcast(
                                r[:, hx], p0rows[("B", gb, h)][:])
                        repBg[gb] = r
                    # remaining phase-A feat producers (DVE)
                    for gg in range(4, NUMS):
                        src = repAg[gg] if gg in GPS_A else repA[gg]
                        f = ftf(f"ftA{gg}")
                        nc.vector.tensor_tensor(f[:], xA[:], src[:], op=mult)
                        ftA[gg] = f
                if g == 1:
                    # phase-B rep prefetch: most on sync (free after ~27us),
                    # g7/g11/g15 on scalar; fully-resident tiles
                    dmaB = [gb for gb in range(1, NUMS) if gb not in GPS_B]
                    repB = {}
                    for gb in dmaB:
                        repB[gb] = repBp.tile([C, PH], BF16, tag="repfB",
                                              bufs=len(dmaB),
                                              name=f"repB{gb}")
                    for gb in dmaB:
                        eng = nc.scalar if gb in (7, 11, 15) else nc.sync
                        eng.dma_start(
                            repB[gb][:],
                            fm_drB[gb:gb + 1, :].broadcast_to((C, PH)))
                    for gb in GPS_B:
                        repB[gb] = repBg[gb]

            # ---- feat producers phase B (head of DVE tail) ----
            ftB = {}
            for g in range(3):
                src = rep0B if g == 0 else repB[g]
                f = ftf(f"ftB{g}")
                nc.vector.tensor_tensor(f[:], xB[:], src[:], op=mult)
                ftB[g] = f

            # ---- drains + stores phase A (chase the PE bank order) ----
            osbA = {(pp, oc): osbp.tile([C, 1024], BF16, tag="osb", bufs=4,
                                        name=f"osbA{pp}_{oc}")
                    for pp in range(2) for oc in range(2)}
            for pb in range(4):
                pp, r = divmod(pb, 2)
                sx = slice(r * PB, (r + 1) * PB)
                nc.scalar.activation(osbA[(pp, 0)][:, sx],
                                     psoA[(pb, 0)][:], ident,
                                     bias=b2_t[:, 0:1])
                nc.vector.tensor_scalar_add(osbA[(pp, 1)][:, sx],
                                            psoA[(pb, 1)][:], b2_t[:, 1:2])
            for pp in range(2):
                px = slice(pp * 1024, (pp + 1) * 1024)
                nc.sync.dma_start(out_d[0:C, px], osbA[(pp, 0)][:])
                nc.scalar.dma_start(out_d[C:OUT, px], osbA[(pp, 1)][:])

            # remaining phase-B feat producers
            for g in range(3, NUMS):
                f = ftf(f"ftB{g}")
                nc.vector.tensor_tensor(f[:], xB[:], repB[g][:], op=mult)
                ftB[g] = f

            # ---- main matmuls phase B ----
            psoB = {(pb, oc): psum(pb, oc, f"psoB{pb}_{oc}")
                    for pb in range(4) for oc in range(2)}
            for g in range(NUMS):
                for pb in range(4):
                    rhs = ftB[g][:, pb * PB:(pb + 1) * PB]
                    for oc in range(2):
                        nc.tensor.matmul(psoB[(pb, oc)][:], w2blk(g, oc),
                                         rhs,
                                         start=(g == 0), stop=(g == NUMS - 1))

            # ---- drains + stores phase B ----
            osbB = {(pp, oc): osbp.tile([C, 1024], BF16, tag="osb", bufs=4,
                                        name=f"osbB{pp}_{oc}")
                    for pp in range(2) for oc in range(2)}
            for pb in range(4):
                pp, r = divmod(pb, 2)
                sx = slice(r * PB, (r + 1) * PB)
                nc.scalar.activation(osbB[(pp, 0)][:, sx],
                                     psoB[(pb, 0)][:], ident,
                                     bias=b2_t[:, 0:1])
                nc.vector.tensor_scalar_add(osbB[(pp, 1)][:, sx],
                                            psoB[(pb, 1)][:], b2_t[:, 1:2])
            for pp in range(2):
                px = slice(PH + pp * 1024, PH + (pp + 1) * 1024)
                nc.sync.dma_start(out_d[0:C, px], osbB[(pp, 0)][:])
                nc.scalar.dma_start(out_d[C:OUT, px], osbB[(pp, 1)][:])

    nc.compile()
    return nc


def _prep_params(W1, b1, W2, b2):
    bf = ml_dtypes.bfloat16
    w1s = np.zeros((C, 256), dtype=bf)
    for g in range(NUMS):
        w1s[g * HEADS:(g + 1) * HEADS, g] = W1[g].astype(bf)
    w2t = (
        np.asarray(W2, dtype=np.float32)
        .reshape(2, C, NUMS, C)          # [oc, m, g, k]
        .transpose(3, 2, 0, 1)           # [k, g, oc, m]
        .reshape(C, NUMS * OUT)
        .astype(bf)
    )
    b1c = np.zeros((NUMS, 128), dtype=np.float32)
    b1c[:, 0] = np.asarray(b1, dtype=np.float32)
    b2c = np.zeros((C, 128), dtype=np.float32)
    b2c[:, 0:2] = np.asarray(b2, dtype=np.float32).reshape(2, C).T
    return w1s, w2t, b1c, b2c


def kernel(x, W1, b1, W2, b2, _trace=False, _trace_kwargs=None):
    if "nc" not in _CACHE:
        _CACHE["nc"] = _build()
    nc = _CACHE["nc"]

    w1s, w2t, b1c, b2c = _prep_params(W1, b1, W2, b2)
    xs = np.ascontiguousarray(
        np.asarray(x, dtype=np.float32).reshape(B, C, P).astype(ml_dtypes.bfloat16))
    in_maps = [
        {"x": xs[b_], "w1s": w1s, "w2t": w2t, "b1c": b1c, "b2c": b2c}
        for b_ in range(N_CORES)
    ]
    kwargs = {}
    if _trace:
        kwargs["trace"] = True
        kwargs.update(_trace_kwargs or {})
    res = run_bass_kernel_spmd(nc, in_maps, core_ids=list(range(N_CORES)),
                               **kwargs)
    out = np.stack([np.asarray(res.results[b_]["out"], dtype=np.float32)
                    for b_ in range(N_CORES)])
    out = out.reshape(B, OUT, H, W)
    if _trace:
        _CACHE["last_result"] = res
    return out
